# revision 1
# baseline (speedup 1.0000x reference)
"""Trainium2 Bass kernel: prefix-LM causal self-attention (B=4, T=2048, D=1024, H=16).

Sharding: 8 cores = 4 batches x 2 head-groups (8 heads each).  Each core
computes QKV projection, masked attention and the output projection for its
(batch, head-group); the two partial output-projection results per batch are
summed on the host (tensor-parallel unshard).

Mask identity used on device: allowed(q, k) <=> k <= max(q, p-1), i.e.
blocked <=> (k > q) AND (k >= p).  Scores are computed transposed
S^T[k_partition, q_free] so the softmax denominator comes free from a
ones-augmented V column in the P@V matmul, and no row-max subtraction is
needed (|scores| ~ N(0,1) after the 1/sqrt(dk) scale).
"""

import sys

for _p in ("/opt/trn_rl_repo", "/opt/pypackages"):
    if _p not in sys.path:
        sys.path.append(_p)

from contextlib import ExitStack

import numpy as np

import concourse.bass as bass  # noqa: F401
import concourse.tile as tile
from concourse import bacc, mybir
from concourse.bass_utils import run_bass_kernel_spmd

F32 = mybir.dt.float32
F32R = mybir.dt.float32r
EXP = mybir.ActivationFunctionType.Exp
ALU = mybir.AluOpType

B, T, D, H, DK = 4, 2048, 1024, 16, 64
HL = 8            # heads per core
JL = HL * DK      # 512 local attention dims
NCORES = 8
NEG = -1.0e30
SCALE = 0.125     # 1/sqrt(dk)
# Static per-q-tile key extent (prefix_lengths < 1024 by construction).
KMAX = [1024, 1024, 1536, 2048]


def _emit(ctx: ExitStack, tc, y_d, xT_d, wT_d, woT_d, pfx_d):
    nc = tc.nc

    # ---- constants ----------------------------------------------------
    const = ctx.enter_context(tc.tile_pool(name="const", bufs=1))
    ones_t = const.tile([128, 512], F32)
    nc.vector.memset(ones_t, 1.0)
    # tri_all[:, i, :][r, j] = 1.0 where (128*i + r) > j else 0  (k > q)
    tri_all = const.tile([128, 4, 512], F32)
    for i in range(4):
        nc.gpsimd.affine_select(
            tri_all[:, i, :], ones_t, pattern=[[-1, 512]], base=128 * i,
            channel_multiplier=1, compare_op=ALU.is_gt, fill=0.0,
        )
    kio = const.tile([128, 1], F32)
    nc.gpsimd.iota(kio, pattern=[[1, 1]], base=0, channel_multiplier=1,
                   allow_small_or_imprecise_dtypes=True)
    p_one = const.tile([1, 1], F32)
    nc.sync.dma_start(p_one, pfx_d)
    p_bcast = const.tile([128, 1], F32)
    nc.gpsimd.partition_broadcast(p_bcast, p_one)

    # ---- resident activations -----------------------------------------
    qk_pool = ctx.enter_context(tc.tile_pool(name="qk", bufs=1))
    qk_all = qk_pool.tile([128, 8, T], F32R)      # rows: q(0-3) then k(4-7)
    v_pool = ctx.enter_context(tc.tile_pool(name="v", bufs=1))
    v_all = v_pool.tile([128, 16, HL * 65], F32R)  # per head 64 v-dims + ones col
    # set the per-head ones columns (f32r memset fails the ISA check, so
    # bit-copy 1.0f from the f32 ones tile instead)
    ones_cols = v_all.rearrange("p t (h e) -> p t h e", e=65)[:, :, :, 64:65]
    ones_src = ones_t[:, 0:128].bitcast(F32R).rearrange(
        "p (a b c) -> p a b c", a=16, b=HL)
    nc.vector.tensor_copy(ones_cols, ones_src)

    # ---- phase 1: QKV projection --------------------------------------
    with ExitStack() as p1:
        w_pool = p1.enter_context(tc.tile_pool(name="w", bufs=1))
        w_tiles = [w_pool.tile([128, 3 * JL], F32R, name=f"w{c}", tag=f"w{c}")
                   for c in range(8)]
        x_pool = p1.enter_context(tc.tile_pool(name="xt", bufs=9))
        ps1 = p1.enter_context(tc.tile_pool(name="ps1", bufs=4, space="PSUM"))

        for tt in range(4):
            t0 = tt * 512
            xts = []
            for c in range(8):
                xt = x_pool.tile([128, 512], F32R, tag="xt")
                nc.sync.dma_start(xt, xT_d[c * 128:(c + 1) * 128, t0:t0 + 512])
                xts.append(xt)
            if tt == 0:
                # weights stream in behind the first x tiles
                for c in range(8):
                    nc.sync.dma_start(w_tiles[c],
                                      wT_d[c * 128:(c + 1) * 128, :])
            # Q^T and K^T (output-transposed GEMM)
            for ot in range(8):
                ps = ps1.tile([128, 512], F32, tag="ps")
                for c in range(8):
                    nc.tensor.matmul(ps, w_tiles[c][:, ot * 128:(ot + 1) * 128],
                                     xts[c], start=(c == 0), stop=(c == 7))
                nc.vector.tensor_copy(qk_all[:, ot, t0:t0 + 512], ps.bitcast(F32R))
            # V (natural layout GEMM), strided into 65-wide head blocks
            for st in range(4):
                kt = tt * 4 + st
                ps = ps1.tile([128, 512], F32, tag="ps")
                for c in range(8):
                    nc.tensor.matmul(ps, xts[c][:, st * 128:(st + 1) * 128],
                                     w_tiles[c][:, 2 * JL:3 * JL],
                                     start=(c == 0), stop=(c == 7))
                dst = v_all[:, kt, :].rearrange("p (h e) -> p h e", h=HL)[:, :, 0:64]
                nc.vector.tensor_copy(dst, ps.bitcast(F32R).rearrange("p (h e) -> p h e", h=HL))

    # ---- phase 2: attention -------------------------------------------
    # allocated after phase-1 pools close so it reuses the w/xt space
    oT_pool = ctx.enter_context(tc.tile_pool(name="ot", bufs=1))
    oT_all = oT_pool.tile([128, 4, T], F32R)
    with ExitStack() as p2:
        bias_pool = p2.enter_context(tc.tile_pool(name="bias", bufs=1))
        pv_pool = p2.enter_context(tc.tile_pool(name="pv", bufs=12))
        e_pool = p2.enter_context(tc.tile_pool(name="e", bufs=6))
        dn_pool = p2.enter_context(tc.tile_pool(name="dn", bufs=2))
        rb_pool = p2.enter_context(tc.tile_pool(name="rb", bufs=2))
        s_psum = p2.enter_context(tc.tile_pool(name="ps2", bufs=3, space="PSUM"))
        o_psum = p2.enter_context(tc.tile_pool(name="po", bufs=1, space="PSUM"))

        for qt in range(4):
            q0 = qt * 512
            nk = KMAX[qt] // 128
            # per-(k-row-tile) prefix bias vectors: (row_k >= p) * NEG
            k0s = [q0 + 128 * i for i in range(4)]
            if qt == 0:
                k0s += [512, 640, 768, 896]
            pb = {}
            for k0 in k0s:
                pk = pv_pool.tile([128, 1], F32, tag="pk")
                nc.vector.tensor_scalar_add(pk, p_bcast, float(-k0))  # p - k0
                pv = pv_pool.tile([128, 1], F32, tag="pv")
                nc.vector.tensor_scalar(pv, kio, pk, NEG, ALU.is_ge, ALU.mult)
                pb[k0] = pv
            # combined diagonal bias tiles: tri(k>q) * (k>=p)*NEG
            bias_all = bias_pool.tile([128, 4, 512], F32, tag="bias")
            for i in range(4):
                nc.vector.tensor_scalar_mul(bias_all[:, i, :], tri_all[:, i, :],
                                            pb[q0 + 128 * i])

            # software-pipelined over a flat (hp, ki) step list: S matmuls
            # are issued PIPE steps ahead of the mask/exp/V consumers so the
            # in-order PE queue never stalls on the DVE/ACT exp chain.
            def kt_order(nk):
                # diagonal (DVE-masked) k-tiles sit at rel 0..384; spread
                # them out so their longer mask+exp chains are hidden by
                # cheap below/above-diagonal steps
                diag = [k for k in range(nk) if 0 <= k * 128 - q0 <= 384]
                rest = [k for k in range(nk) if k not in diag]
                out, di, ri = [], 0, 0
                for j in range(nk):
                    if j % 4 == 0 and di < len(diag):
                        out.append(diag[di]); di += 1
                    elif ri < len(rest):
                        out.append(rest[ri]); ri += 1
                    else:
                        out.append(diag[di]); di += 1
                return out

            korder = kt_order(nk)
            steps = [(hp, ki) for hp in range(4) for ki in korder]
            PIPE = 2
            live = {}
            otiles = {}

            def emit_s(j):
                hp, ki = steps[j]
                k0 = ki * 128
                sps = s_psum.tile([128, 2, 512], F32, tag="s")
                qa = qk_all[0:64, hp, q0:q0 + 512]
                qb = qk_all[64:128, hp, q0:q0 + 512]
                ka = qk_all[0:64, 4 + hp, k0:k0 + 128]
                kb = qk_all[64:128, 4 + hp, k0:k0 + 128]
                nc.tensor.matmul(sps[:, 0, :], ka, qa, start=True, stop=True)
                nc.tensor.matmul(sps[:, 1, :], kb, qb, start=True, stop=True)
                live[j] = sps

            def emit_v(j):
                hp, ki = steps[j]
                pos = j % nk
                hA, hB = 2 * hp, 2 * hp + 1
                k0 = ki * 128
                rel = k0 - q0
                sps = live.pop(j)
                et = e_pool.tile([128, 2, 512], F32R, tag="e")
                if 0 <= rel <= 384:
                    bb = bias_all[:, rel // 128, :].unsqueeze(1)
                    bb = bb.broadcast_to([128, 2, 512])
                    nc.vector.tensor_tensor(sps, sps, bb, ALU.add)
                    nc.scalar.activation(et, sps, EXP, scale=SCALE)
                elif rel > 384:      # fully above diagonal: prefix-only rows
                    nc.scalar.activation(et, sps, EXP, bias=pb[k0], scale=SCALE)
                else:                # fully below diagonal: all allowed
                    nc.scalar.activation(et, sps, EXP, scale=SCALE)
                if pos == 0:
                    otiles[hp] = (
                        o_psum.tile([65, 512], F32, tag="oa", name=f"oa{hp}"),
                        o_psum.tile([65, 512], F32, tag="ob", name=f"ob{hp}"),
                    )
                oA, oB = otiles[hp]
                va = v_all[:, ki, hA * 65:hA * 65 + 65]
                vb = v_all[:, ki, hB * 65:hB * 65 + 65]
                nc.tensor.matmul(oA, va, et[:, 0, :],
                                 start=(pos == 0), stop=(pos == nk - 1),
                                 skip_group_check=True)
                nc.tensor.matmul(oB, vb, et[:, 1, :],
                                 start=(pos == 0), stop=(pos == nk - 1),
                                 skip_group_check=True)
                if pos == nk - 1:
                    # normalize by the ones-row denominator, evict to O^T
                    dn = dn_pool.tile([1, 1024], F32, tag="dn")
                    nc.vector.tensor_copy(dn[0:1, 0:512], oA[64:65, :])
                    nc.vector.tensor_copy(dn[0:1, 512:1024], oB[64:65, :])
                    rv = dn_pool.tile([1, 1024], F32, tag="rv")
                    nc.vector.reciprocal_approx_fast(out=rv, in_=dn)
                    rb = rb_pool.tile([128, 1024], F32, tag="rb")
                    nc.gpsimd.partition_broadcast(rb, rv)
                    nc.vector.tensor_tensor(oT_all[0:64, hp, q0:q0 + 512],
                                            oA[0:64, :], rb[0:64, 0:512],
                                            ALU.mult)
                    nc.vector.tensor_tensor(oT_all[64:128, hp, q0:q0 + 512],
                                            oB[0:64, :], rb[64:128, 512:1024],
                                            ALU.mult)

            for j in range(len(steps) + PIPE):
                if j < len(steps):
                    emit_s(j)
                if j >= PIPE:
                    emit_v(j - PIPE)

    # ---- phase 3: output projection -----------------------------------
    with ExitStack() as p3:
        wo_pool = p3.enter_context(tc.tile_pool(name="wo", bufs=1))
        wo_all = wo_pool.tile([128, 4, D], F32R)
        for jc in range(4):
            nc.sync.dma_start(wo_all[:, jc, :], woT_d[jc * 128:(jc + 1) * 128, :])
        y_pool = p3.enter_context(tc.tile_pool(name="ysb", bufs=4))
        ps3 = p3.enter_context(tc.tile_pool(name="ps3", bufs=4, space="PSUM"))
        for ttt in range(16):
            t0 = ttt * 128
            for ob in range(2):
                ps = ps3.tile([128, 512], F32, tag="y")
                for jc in range(4):
                    nc.tensor.matmul(ps, oT_all[:, jc, t0:t0 + 128],
                                     wo_all[:, jc, ob * 512:(ob + 1) * 512],
                                     start=(jc == 0), stop=(jc == 3))
                ysb = y_pool.tile([128, 512], F32, tag="ysb")
                nc.vector.tensor_copy(ysb, ps)
                nc.sync.dma_start(y_d[t0:t0 + 128, ob * 512:(ob + 1) * 512], ysb)


def build_module():
    nc = bacc.Bacc("TRN2", target_bir_lowering=False, debug=False,
                   num_devices=NCORES)
    xT_d = nc.dram_tensor("xT", [D, T], F32R, kind="ExternalInput").ap()
    wT_d = nc.dram_tensor("wT", [D, 3 * JL], F32R, kind="ExternalInput").ap()
    woT_d = nc.dram_tensor("woT", [JL, D], F32R, kind="ExternalInput").ap()
    pfx_d = nc.dram_tensor("pfx", [1, 1], F32, kind="ExternalInput").ap()
    y_d = nc.dram_tensor("y", [T, D], F32, kind="ExternalOutput").ap()
    with tile.TileContext(nc) as tc:
        with ExitStack() as ctx:
            _emit(ctx, tc, y_d, xT_d, wT_d, woT_d, pfx_d)
    nc.compile()
    return nc


_NC = None


def _get_nc():
    global _NC
    if _NC is None:
        _NC = build_module()
    return _NC


def shard_inputs(x, prefix_lengths, W_qkv, W_o):
    x = np.asarray(x, dtype=np.float32)
    W_qkv = np.asarray(W_qkv, dtype=np.float32)
    W_o = np.asarray(W_o, dtype=np.float32)
    pl = np.asarray(prefix_lengths)
    in_maps = []
    for c in range(NCORES):
        b, g = c // 2, c % 2
        W_loc = np.concatenate([
            W_qkv[JL * g:JL * (g + 1)],
            W_qkv[D + JL * g:D + JL * (g + 1)],
            W_qkv[2 * D + JL * g:2 * D + JL * (g + 1)],
        ], axis=0)
        p = float(min(max(int(pl[b]), 0), T))
        in_maps.append({
            "xT": np.ascontiguousarray(x[b].T),
            "wT": np.ascontiguousarray(W_loc.T),
            "woT": np.ascontiguousarray(W_o[:, JL * g:JL * (g + 1)].T),
            "pfx": np.array([[p]], dtype=np.float32),
        })
    return in_maps


def run(x, prefix_lengths, W_qkv, W_o, **kw):
    """Run the kernel; returns (y, BassKernelResults)."""
    nc = _get_nc()
    in_maps = shard_inputs(x, prefix_lengths, W_qkv, W_o)
    res = run_bass_kernel_spmd(nc, in_maps, core_ids=list(range(NCORES)), **kw)
    y = np.zeros((B, T, D), dtype=np.float32)
    for c in range(NCORES):
        y[c // 2] += res.results[c]["y"]
    return y, res


def kernel(x, prefix_lengths, W_qkv, W_o):
    y, _ = run(x, prefix_lengths, W_qkv, W_o)
    return y



# revision 6
# speedup vs baseline: 1.1993x; 1.1993x over previous
"""Trainium2 Bass kernel: prefix-LM causal self-attention (B=4, T=2048, D=1024, H=16).

Sharding: 8 cores = 4 batches x 2 head-groups (8 heads each).  Each core
computes QKV projection, masked attention and the output projection for its
(batch, head-group); the two partial output-projection results per batch are
summed on the host (tensor-parallel unshard).

v2 vs baseline:
  - bf16 activations/weights everywhere (rel err ~6e-3, tolerance 2e-2).
  - x and W fully resident in SBUF; DMA chunked+interleaved so the PE
    starts ~3us in and never starves (keeps the 2.4GHz p-state).
  - Diagonal-tile mask bias is pre-seeded into PSUM (gpsimd) before the
    S matmul accumulates on top, so the S->exp->PV chain never waits on
    a DVE mask add.
  - Phases are interleaved: attention for q-tile qt starts as soon as
    its K/V rows exist (after phase-1 tt<=1); phase-1 tt2/tt3 and the
    phase-3 output projection are spread between attention steps as PE
    filler so the exp (ACT) stream stays hidden under PE work.

Mask identity: allowed(q, k) <=> k <= max(q, p-1), i.e. blocked <=>
(k > q) AND (k >= p).  Scores are computed transposed S^T[k, q] so the
softmax denominator comes free from a ones-augmented V column in the P@V
matmul (no row-max subtraction needed; |scores| ~ N(0,1)).
"""

import sys

for _p in ("/opt/trn_rl_repo", "/opt/pypackages"):
    if _p not in sys.path:
        sys.path.append(_p)

from contextlib import ExitStack

import numpy as np

import concourse.bass as bass  # noqa: F401
import concourse.tile as tile
from concourse import bacc, mybir
from concourse.bass_utils import run_bass_kernel_spmd

F32 = mybir.dt.float32
BF16 = mybir.dt.bfloat16
EXP = mybir.ActivationFunctionType.Exp
ALU = mybir.AluOpType

B, T, D, H, DK = 4, 2048, 1024, 16, 64
HL = 8            # heads per core
JL = HL * DK      # 512 local attention dims
NCORES = 8
NEG = -1.0e30
SCALE = 0.125     # 1/sqrt(dk)
# Static per-q-tile key extent (prefix_lengths < 1024 by construction).
KMAX = [1024, 1024, 1536, 2048]
SEED_ENGINE = "vector"   # engine for diag-tile psum bias seeds (gpsimd can't touch PSUM)


def _emit(ctx: ExitStack, tc, y_d, xT_d, wT_d, woT_d, pfx_d):
    nc = tc.nc
    seed_eng = getattr(nc, SEED_ENGINE)

    # ---- constants ----------------------------------------------------
    const = ctx.enter_context(tc.tile_pool(name="const", bufs=1))
    ones_t = const.tile([128, 512], F32)
    nc.vector.memset(ones_t, 1.0)
    # tri_all[:, i, :][r, j] = 1.0 where (128*i + r) > j else 0  (k > q)
    tri_all = const.tile([128, 4, 512], F32)
    for i in range(4):
        nc.gpsimd.affine_select(
            tri_all[:, i, :], ones_t, pattern=[[-1, 512]], base=128 * i,
            channel_multiplier=1, compare_op=ALU.is_gt, fill=0.0,
        )
    kio = const.tile([128, 1], F32)
    nc.gpsimd.iota(kio, pattern=[[1, 1]], base=0, channel_multiplier=1,
                   allow_small_or_imprecise_dtypes=True)
    p_one = const.tile([1, 1], F32)
    nc.sync.dma_start(p_one, pfx_d)
    p_bcast = const.tile([128, 1], F32)
    nc.gpsimd.partition_broadcast(p_bcast, p_one)

    # ---- resident tensors ---------------------------------------------
    big = ctx.enter_context(tc.tile_pool(name="big", bufs=1))
    xt_all = big.tile([128, 8, T], BF16)          # x^T, chunked by c
    w_all = big.tile([128, 8, 3 * JL], BF16)      # W_qkv^T, chunked by c
    qk_all = big.tile([128, 8, T], BF16)          # rows: q(0-3) then k(4-7)
    v_all = big.tile([128, 16, HL * 65], BF16)    # per head 64 v-dims + ones col
    wo_all = big.tile([128, 4, D], BF16)          # W_o^T, chunked by jc
    oT_all = big.tile([128, 4, T], BF16)          # attention out^T per hp

    # ones columns for the softmax denominator
    ones_cols = v_all.rearrange("p t (h e) -> p t h e", e=65)[:, :, :, 64:65]
    nc.vector.memset(ones_cols, 1.0)

    # ---- input DMAs (ordered: w/x interleaved per c, then wo) ---------
    for c in range(8):
        nc.sync.dma_start(w_all[:, c, :], wT_d[c * 128:(c + 1) * 128, :])
        nc.sync.dma_start(xt_all[:, c, :], xT_d[c * 128:(c + 1) * 128, :])
    for jc in range(4):
        nc.sync.dma_start(wo_all[:, jc, :], woT_d[jc * 128:(jc + 1) * 128, :])

    # ---- shared pools --------------------------------------------------
    psf = ctx.enter_context(tc.tile_pool(name="psf", bufs=2, space="PSUM"))
    s_psum = ctx.enter_context(tc.tile_pool(name="ps2", bufs=2, space="PSUM"))
    o_psum = ctx.enter_context(tc.tile_pool(name="po", bufs=1, space="PSUM"))
    evict_pool = ctx.enter_context(tc.tile_pool(name="ev", bufs=4))
    bias_pool = ctx.enter_context(tc.tile_pool(name="bias", bufs=2))
    pv_pool = ctx.enter_context(tc.tile_pool(name="pv", bufs=12))
    e_pool = ctx.enter_context(tc.tile_pool(name="e", bufs=4))
    dn_pool = ctx.enter_context(tc.tile_pool(name="dn", bufs=2))
    rb_pool = ctx.enter_context(tc.tile_pool(name="rb", bufs=2))
    y_pool = ctx.enter_context(tc.tile_pool(name="ysb", bufs=4))

    # ---- phase-1 group emitters ---------------------------------------
    def _evict(eng, dst, src):
        if eng is nc.scalar:
            eng.copy(dst, src)
        else:
            eng.tensor_copy(dst, src)

    def p1_qk_group(tt, ot, eng):
        t0 = tt * 512
        ps = psf.tile([128, 512], F32, tag="psf")
        for c in range(8):
            nc.tensor.matmul(ps, w_all[:, c, ot * 128:(ot + 1) * 128],
                             xt_all[:, c, t0:t0 + 512],
                             start=(c == 0), stop=(c == 7))
        _evict(eng, qk_all[:, ot, t0:t0 + 512], ps)

    def p1_v_group(tt, st, eng):
        kt = tt * 4 + st
        t0 = tt * 512 + st * 128
        ps = psf.tile([128, 512], F32, tag="psf")
        for c in range(8):
            nc.tensor.matmul(ps, xt_all[:, c, t0:t0 + 128],
                             w_all[:, c, 2 * JL:3 * JL],
                             start=(c == 0), stop=(c == 7))
        dst = v_all[:, kt, :].rearrange("p (h e) -> p h e", h=HL)[:, :, 0:64]
        _evict(eng, dst, ps.rearrange("p (h e) -> p h e", h=HL))

    def p1_groups(tt, evict_engines):
        out = []
        for ot in range(8):
            eng = evict_engines[ot % len(evict_engines)]
            out.append(lambda ot=ot, eng=eng: p1_qk_group(tt, ot, eng))
        for st in range(4):
            eng = evict_engines[st % len(evict_engines)]
            out.append(lambda st=st, eng=eng: p1_v_group(tt, st, eng))
        return out

    # ---- phase-3 group emitters ---------------------------------------
    def p3_group(ttt, ob):
        t0 = ttt * 128
        ps = psf.tile([128, 512], F32, tag="psf")
        for jc in range(4):
            nc.tensor.matmul(ps, oT_all[:, jc, t0:t0 + 128],
                             wo_all[:, jc, ob * 512:(ob + 1) * 512],
                             start=(jc == 0), stop=(jc == 3))
        ysb = y_pool.tile([128, 512], F32, tag="ysb")
        nc.vector.tensor_copy(ysb, ps)
        nc.sync.dma_start(y_d[t0:t0 + 128, ob * 512:(ob + 1) * 512], ysb)

    def p3_groups(qt):
        return [lambda ttt=qt * 4 + i, ob=ob: p3_group(ttt, ob)
                for i in range(4) for ob in range(2)]

    # ---- attention: one q-tile, with PE filler interleaved ------------
    def run_qt(qt, fillers):
        q0 = qt * 512
        nk = KMAX[qt] // 128
        # per-(k-row-tile) prefix bias vectors: (row_k >= p) * NEG
        k0s = [q0 + 128 * i for i in range(4)]
        if qt == 0:
            k0s += [512, 640, 768, 896]
        pb = {}
        for k0 in k0s:
            pk = pv_pool.tile([128, 1], F32, tag="pk")
            nc.vector.tensor_scalar_add(pk, p_bcast, float(-k0))  # p - k0
            pv = pv_pool.tile([128, 1], F32, tag="pv")
            nc.vector.tensor_scalar(pv, kio, pk, NEG, ALU.is_ge, ALU.mult)
            pb[k0] = pv
        # combined diagonal bias tiles: tri(k>q) * (k>=p)*NEG
        bias_all = bias_pool.tile([128, 4, 512], F32, tag="bias")
        for i in range(4):
            nc.vector.tensor_scalar_mul(bias_all[:, i, :], tri_all[:, i, :],
                                        pb[q0 + 128 * i])

        # spread diagonal (seeded) k-tiles out in the step order
        def kt_order(nk):
            diag = [k for k in range(nk) if 0 <= k * 128 - q0 <= 384]
            rest = [k for k in range(nk) if k not in diag]
            out, di, ri = [], 0, 0
            for j in range(nk):
                if j % 4 == 0 and di < len(diag):
                    out.append(diag[di]); di += 1
                elif ri < len(rest):
                    out.append(rest[ri]); ri += 1
                else:
                    out.append(diag[di]); di += 1
            return out

        korder = kt_order(nk)
        steps = [(hp, ki) for hp in range(4) for ki in korder]
        PIPE = 2
        live = {}
        otiles = {}

        def emit_s(j):
            hp, ki = steps[j]
            k0 = ki * 128
            rel = k0 - q0
            sps = s_psum.tile([128, 2, 512], F32, tag="s")
            seeded = 0 <= rel <= 384
            if seeded:
                bb = bias_all[:, rel // 128, :].unsqueeze(1)
                seed_eng.tensor_copy(sps, bb.broadcast_to([128, 2, 512]))
            qa = qk_all[0:64, hp, q0:q0 + 512]
            qb = qk_all[64:128, hp, q0:q0 + 512]
            ka = qk_all[0:64, 4 + hp, k0:k0 + 128]
            kb = qk_all[64:128, 4 + hp, k0:k0 + 128]
            nc.tensor.matmul(sps[:, 0, :], ka, qa, start=not seeded, stop=True,
                             skip_group_check=True)
            nc.tensor.matmul(sps[:, 1, :], kb, qb, start=not seeded, stop=True,
                             skip_group_check=True)
            live[j] = sps

        def emit_v(j):
            hp, ki = steps[j]
            pos = j % nk
            hA, hB = 2 * hp, 2 * hp + 1
            k0 = ki * 128
            rel = k0 - q0
            sps = live.pop(j)
            et = e_pool.tile([128, 2, 512], BF16, tag="e")
            if rel > 384:        # fully above diagonal: prefix-only rows
                nc.scalar.activation(et, sps, EXP, bias=pb[k0], scale=SCALE)
            else:                # diag (pre-seeded) or below: plain exp
                nc.scalar.activation(et, sps, EXP, scale=SCALE)
            if pos == 0:
                otiles[hp] = (
                    o_psum.tile([65, 512], F32, tag="oa", name=f"oa{hp}"),
                    o_psum.tile([65, 512], F32, tag="ob", name=f"ob{hp}"),
                )
            oA, oB = otiles[hp]
            va = v_all[:, ki, hA * 65:hA * 65 + 65]
            vb = v_all[:, ki, hB * 65:hB * 65 + 65]
            nc.tensor.matmul(oA, va, et[:, 0, :],
                             start=(pos == 0), stop=(pos == nk - 1),
                             skip_group_check=True)
            nc.tensor.matmul(oB, vb, et[:, 1, :],
                             start=(pos == 0), stop=(pos == nk - 1),
                             skip_group_check=True)
            if pos == nk - 1:
                # normalize by the ones-row denominator, evict to O^T
                dn = dn_pool.tile([1, 1024], F32, tag="dn")
                nc.vector.tensor_copy(dn[0:1, 0:512], oA[64:65, :])
                nc.vector.tensor_copy(dn[0:1, 512:1024], oB[64:65, :])
                rv = dn_pool.tile([1, 1024], F32, tag="rv")
                nc.vector.reciprocal_approx_fast(out=rv, in_=dn)
                rb = rb_pool.tile([128, 1024], F32, tag="rb")
                nc.gpsimd.partition_broadcast(rb, rv)
                nc.vector.tensor_tensor(oT_all[0:64, hp, q0:q0 + 512],
                                        oA[0:64, :], rb[0:64, 0:512],
                                        ALU.mult)
                nc.vector.tensor_tensor(oT_all[64:128, hp, q0:q0 + 512],
                                        oB[0:64, :], rb[64:128, 512:1024],
                                        ALU.mult)

        # filler positions: spread evenly across the step list
        nf = len(fillers)
        fill_at = {}
        if nf:
            for fi in range(nf):
                pos = 1 + (fi * len(steps)) // nf
                fill_at.setdefault(pos, []).append(fillers[fi])

        for j in range(len(steps) + PIPE):
            if j >= PIPE:
                emit_v(j - PIPE)
            for f in fill_at.get(j, ()):
                f()
            if j < len(steps):
                emit_s(j)

    # ---- schedule ------------------------------------------------------
    # seg1: phase-1 tt0, tt1 (evictions split DVE/ACT)
    for g in p1_groups(0, [nc.vector, nc.scalar]):
        g()
    for g in p1_groups(1, [nc.vector, nc.scalar]):
        g()
    # seg2: qt0 attention, phase-1 tt2 as filler
    run_qt(0, p1_groups(2, [nc.vector]))
    # seg3: qt1 attention, phase-1 tt3 as filler
    run_qt(1, p1_groups(3, [nc.vector]))
    # seg4: qt2 attention, phase-3 for qt0/qt1 as filler
    run_qt(2, p3_groups(0) + p3_groups(1))
    # seg5: qt3 attention, phase-3 for qt2 as filler
    run_qt(3, p3_groups(2))
    # seg6: phase-3 for qt3
    for g in p3_groups(3):
        g()


def build_module():
    nc = bacc.Bacc("TRN2", target_bir_lowering=False, debug=False,
                   num_devices=NCORES)
    xT_d = nc.dram_tensor("xT", [D, T], BF16, kind="ExternalInput").ap()
    wT_d = nc.dram_tensor("wT", [D, 3 * JL], BF16, kind="ExternalInput").ap()
    woT_d = nc.dram_tensor("woT", [JL, D], BF16, kind="ExternalInput").ap()
    pfx_d = nc.dram_tensor("pfx", [1, 1], F32, kind="ExternalInput").ap()
    y_d = nc.dram_tensor("y", [T, D], F32, kind="ExternalOutput").ap()
    with tile.TileContext(nc) as tc:
        with ExitStack() as ctx:
            _emit(ctx, tc, y_d, xT_d, wT_d, woT_d, pfx_d)
    nc.compile()
    return nc


_NC = None


def _get_nc():
    global _NC
    if _NC is None:
        _NC = build_module()
    return _NC


def _bf16(a):
    import ml_dtypes
    return np.ascontiguousarray(a, dtype=np.float32).astype(ml_dtypes.bfloat16)


def shard_inputs(x, prefix_lengths, W_qkv, W_o):
    x = np.asarray(x, dtype=np.float32)
    W_qkv = np.asarray(W_qkv, dtype=np.float32)
    W_o = np.asarray(W_o, dtype=np.float32)
    pl = np.asarray(prefix_lengths)
    in_maps = []
    for c in range(NCORES):
        b, g = c // 2, c % 2
        W_loc = np.concatenate([
            W_qkv[JL * g:JL * (g + 1)],
            W_qkv[D + JL * g:D + JL * (g + 1)],
            W_qkv[2 * D + JL * g:2 * D + JL * (g + 1)],
        ], axis=0)
        p = float(min(max(int(pl[b]), 0), T))
        in_maps.append({
            "xT": _bf16(x[b].T),
            "wT": _bf16(W_loc.T),
            "woT": _bf16(W_o[:, JL * g:JL * (g + 1)].T),
            "pfx": np.array([[p]], dtype=np.float32),
        })
    return in_maps


def run(x, prefix_lengths, W_qkv, W_o, **kw):
    """Run the kernel; returns (y, BassKernelResults)."""
    nc = _get_nc()
    in_maps = shard_inputs(x, prefix_lengths, W_qkv, W_o)
    res = run_bass_kernel_spmd(nc, in_maps, core_ids=list(range(NCORES)), **kw)
    y = np.zeros((B, T, D), dtype=np.float32)
    for c in range(NCORES):
        y[c // 2] += res.results[c]["y"]
    return y, res


def kernel(x, prefix_lengths, W_qkv, W_o):
    y, _ = run(x, prefix_lengths, W_qkv, W_o)
    return y


# revision 16
# speedup vs baseline: 1.3528x; 1.1280x over previous
"""Trainium2 Bass kernel: prefix-LM causal self-attention (B=4, T=2048, D=1024, H=16).

Sharding: 8 cores = 4 batches x 2 head-groups (8 heads each).  Each core
computes QKV projection, masked attention and the output projection for its
(batch, head-group); the two partial output-projection results per batch are
summed on the host (tensor-parallel unshard).

v2 vs baseline:
  - bf16 activations/weights everywhere (rel err ~6e-3, tolerance 2e-2).
  - x and W fully resident in SBUF; DMA chunked+interleaved so the PE
    starts ~3us in and never starves (keeps the 2.4GHz p-state).
  - Diagonal-tile mask bias is pre-seeded into PSUM (gpsimd) before the
    S matmul accumulates on top, so the S->exp->PV chain never waits on
    a DVE mask add.
  - Phases are interleaved: attention for q-tile qt starts as soon as
    its K/V rows exist (after phase-1 tt<=1); phase-1 tt2/tt3 and the
    phase-3 output projection are spread between attention steps as PE
    filler so the exp (ACT) stream stays hidden under PE work.

Mask identity: allowed(q, k) <=> k <= max(q, p-1), i.e. blocked <=>
(k > q) AND (k >= p).  Scores are computed transposed S^T[k, q] so the
softmax denominator comes free from a ones-augmented V column in the P@V
matmul (no row-max subtraction needed; |scores| ~ N(0,1)).
"""

import sys

for _p in ("/opt/trn_rl_repo", "/opt/pypackages"):
    if _p not in sys.path:
        sys.path.append(_p)

from contextlib import ExitStack

import numpy as np

import concourse.bass as bass  # noqa: F401
import concourse.tile as tile
from concourse import bacc, mybir
from concourse.bass_utils import run_bass_kernel_spmd

F32 = mybir.dt.float32
BF16 = mybir.dt.bfloat16
EXP = mybir.ActivationFunctionType.Exp
ALU = mybir.AluOpType

B, T, D, H, DK = 4, 2048, 1024, 16, 64
HL = 8            # heads per core
JL = HL * DK      # 512 local attention dims
NCORES = 8
NEG = -1.0e30
SCALE = 0.125     # 1/sqrt(dk)
# Static per-q-tile key extent (prefix_lengths < 1024 by construction).
KMAX = [1024, 1024, 1536, 2048]
def _emit(ctx: ExitStack, tc, y_d, xT_d, wT_d, woT_d, pfx_d):
    nc = tc.nc

    # ---- constants ----------------------------------------------------
    const = ctx.enter_context(tc.tile_pool(name="const", bufs=1))
    ones_t = const.tile([128, 512], F32)
    nc.vector.memset(ones_t, 1.0)
    # tri_all[:, i, :][r, j] = 1.0 where (128*i + r) > j else 0  (k > q)
    tri_all = const.tile([128, 4, 512], F32)
    for i in range(4):
        nc.gpsimd.affine_select(
            tri_all[:, i, :], ones_t, pattern=[[-1, 512]], base=128 * i,
            channel_multiplier=1, compare_op=ALU.is_gt, fill=0.0,
        )
    kio = const.tile([128, 1], F32)
    nc.gpsimd.iota(kio, pattern=[[1, 1]], base=0, channel_multiplier=1,
                   allow_small_or_imprecise_dtypes=True)
    p_one = const.tile([1, 1], F32)
    nc.sync.dma_start(p_one, pfx_d)
    p_bcast = const.tile([128, 1], F32)
    nc.gpsimd.partition_broadcast(p_bcast, p_one)

    # ---- resident tensors ---------------------------------------------
    big = ctx.enter_context(tc.tile_pool(name="big", bufs=1))
    xt_all = big.tile([128, 8, T], BF16)          # x^T, chunked by c
    w_all = big.tile([128, 8, 3 * JL], BF16)      # W_qkv^T, chunked by c
    qk_all = big.tile([128, 8, T], BF16)          # rows: q(0-3) then k(4-7)
    v_all = big.tile([128, 16, HL * 65], BF16)    # per head 64 v-dims + ones col
    wo_all = big.tile([128, 4, D], BF16)          # W_o^T, chunked by jc
    oT_all = big.tile([128, 4, T], BF16)          # attention out^T per hp

    # ones columns for the softmax denominator
    ones_cols = v_all.rearrange("p t (h e) -> p t h e", e=65)[:, :, :, 64:65]
    nc.vector.memset(ones_cols, 1.0)

    # ---- input DMAs (ordered: w/x interleaved per c, then wo) ---------
    for c in range(8):
        nc.sync.dma_start(w_all[:, c, :], wT_d[c * 128:(c + 1) * 128, :])
        nc.sync.dma_start(xt_all[:, c, :], xT_d[c * 128:(c + 1) * 128, :])
    for jc in range(4):
        nc.sync.dma_start(wo_all[:, jc, :], woT_d[jc * 128:(jc + 1) * 128, :])

    # ---- shared pools --------------------------------------------------
    psf = ctx.enter_context(tc.tile_pool(name="psf", bufs=2, space="PSUM"))
    s_psum = ctx.enter_context(tc.tile_pool(name="ps2", bufs=2, space="PSUM"))
    o_psum = ctx.enter_context(tc.tile_pool(name="po", bufs=1, space="PSUM"))
    evict_pool = ctx.enter_context(tc.tile_pool(name="ev", bufs=4))
    bias_pool = ctx.enter_context(tc.tile_pool(name="bias", bufs=2))
    pv_pool = ctx.enter_context(tc.tile_pool(name="pv", bufs=12))
    e_pool = ctx.enter_context(tc.tile_pool(name="e", bufs=4))
    dn_pool = ctx.enter_context(tc.tile_pool(name="dn", bufs=2))
    rb_pool = ctx.enter_context(tc.tile_pool(name="rb", bufs=2))
    y_pool = ctx.enter_context(tc.tile_pool(name="ysb", bufs=4))

    # ---- phase-1 group emitters ---------------------------------------
    def _evict(eng, dst, src, accum=False):
        if accum:
            nc.vector.tensor_tensor(dst, src, dst, ALU.add)
        elif eng is nc.scalar:
            eng.copy(dst, src)
        else:
            eng.tensor_copy(dst, src)

    def p1_qk_group(tt, ot, eng, cs=(0, 8), accum=False):
        t0 = tt * 512
        c0, c1 = cs
        ps = psf.tile([128, 512], F32, tag="psf")
        for c in range(c0, c1):
            nc.tensor.matmul(ps, w_all[:, c, ot * 128:(ot + 1) * 128],
                             xt_all[:, c, t0:t0 + 512],
                             start=(c == c0), stop=(c == c1 - 1))
        _evict(eng, qk_all[:, ot, t0:t0 + 512], ps, accum)

    def p1_v_group(tt, st, eng, cs=(0, 8), accum=False):
        kt = tt * 4 + st
        t0 = tt * 512 + st * 128
        c0, c1 = cs
        ps = psf.tile([128, 512], F32, tag="psf")
        for c in range(c0, c1):
            nc.tensor.matmul(ps, xt_all[:, c, t0:t0 + 128],
                             w_all[:, c, 2 * JL:3 * JL],
                             start=(c == c0), stop=(c == c1 - 1))
        dst = v_all[:, kt, :].rearrange("p (h e) -> p h e", h=HL)[:, :, 0:64]
        _evict(eng, dst, ps.rearrange("p (h e) -> p h e", h=HL), accum)

    def p1_groups(tt, evict_engines, cs=(0, 8), accum=False):
        out = []
        for ot in range(8):
            eng = evict_engines[ot % len(evict_engines)]
            out.append(lambda ot=ot, eng=eng: p1_qk_group(tt, ot, eng, cs, accum))
        for st in range(4):
            eng = evict_engines[st % len(evict_engines)]
            out.append(lambda st=st, eng=eng: p1_v_group(tt, st, eng, cs, accum))
        return out

    # ---- phase-3 group emitters ---------------------------------------
    def p3_group(ttt, ob):
        t0 = ttt * 128
        ps = psf.tile([128, 512], F32, tag="psf")
        for jc in range(4):
            nc.tensor.matmul(ps, oT_all[:, jc, t0:t0 + 128],
                             wo_all[:, jc, ob * 512:(ob + 1) * 512],
                             start=(jc == 0), stop=(jc == 3))
        ysb = y_pool.tile([128, 512], F32, tag="ysb")
        nc.vector.tensor_copy(ysb, ps)
        nc.sync.dma_start(y_d[t0:t0 + 128, ob * 512:(ob + 1) * 512], ysb)

    def p3_groups(qt):
        return [lambda ttt=qt * 4 + i, ob=ob: p3_group(ttt, ob)
                for i in range(4) for ob in range(2)]

    # ---- attention: one q-tile, with PE filler interleaved ------------
    def run_qt(qt, fillers):
        q0 = qt * 512
        nk = KMAX[qt] // 128
        # per-(k-row-tile) prefix bias vectors: (row_k >= p) * NEG
        k0s = [q0 + 128 * i for i in range(4)]
        if qt == 0:
            k0s += [512, 640, 768, 896]
        pb = {}
        pks = {}
        for k0 in k0s:
            pk = pv_pool.tile([128, 1], F32, tag="pk")
            nc.vector.tensor_scalar_add(pk, p_bcast, float(-k0))  # p - k0
            pv = pv_pool.tile([128, 1], F32, tag="pv")
            nc.vector.tensor_scalar(pv, kio, pk, NEG, ALU.is_ge, ALU.mult)
            pb[k0] = pv
            pks[k0] = pk
        # diagonal 0/1 keep-masks: 1 - tri(k>q)*(k>=p)
        mask_all = bias_pool.tile([128, 4, 512], BF16, tag="mask")
        for i in range(4):
            pm = pv_pool.tile([128, 1], F32, tag="pm")
            nc.vector.tensor_scalar(pm, kio, pks[q0 + 128 * i], -1.0,
                                    ALU.is_ge, ALU.mult)
            nc.vector.tensor_scalar(mask_all[:, i, :], tri_all[:, i, :], pm,
                                    1.0, ALU.mult, ALU.add)

        # spread diagonal (masked) k-tiles out in the step order
        def kt_order(nk):
            diag = [k for k in range(nk) if 0 <= k * 128 - q0 <= 384]
            rest = [k for k in range(nk) if k not in diag]
            out, di, ri = [], 0, 0
            for j in range(nk):
                if j % 4 == 0 and di < len(diag):
                    out.append(diag[di]); di += 1
                elif ri < len(rest):
                    out.append(rest[ri]); ri += 1
                else:
                    out.append(diag[di]); di += 1
            return out

        korder = kt_order(nk)
        steps = [(hp, ki) for hp in range(4) for ki in korder]
        EPIPE = 1   # exp stage lags S by this many steps
        VPIPE = 2   # PV stage lags S
        live = {}
        ets = {}
        otiles = {}

        def emit_s(j):
            hp, ki = steps[j]
            k0 = ki * 128
            sps = s_psum.tile([128, 2, 512], F32, tag="s")
            qa = qk_all[0:64, hp, q0:q0 + 512]
            qb = qk_all[64:128, hp, q0:q0 + 512]
            ka = qk_all[0:64, 4 + hp, k0:k0 + 128]
            kb = qk_all[64:128, 4 + hp, k0:k0 + 128]
            nc.tensor.matmul(sps[:, 0, :], ka, qa, start=True, stop=True)
            nc.tensor.matmul(sps[:, 1, :], kb, qb, start=True, stop=True)
            live[j] = sps

        def emit_e(j):
            hp, ki = steps[j]
            k0 = ki * 128
            rel = k0 - q0
            sps = live.pop(j)
            et = e_pool.tile([128, 2, 512], BF16, tag="e")
            if rel > 384:        # fully above diagonal: prefix-only rows
                nc.scalar.activation(et, sps, EXP, bias=pb[k0], scale=SCALE)
            else:
                nc.scalar.activation(et, sps, EXP, scale=SCALE)
            if 0 <= rel <= 384:  # diagonal: zero the blocked region post-exp
                mm = mask_all[:, rel // 128, :].unsqueeze(1)
                em = e_pool.tile([128, 2, 512], BF16, tag="em")
                nc.vector.tensor_tensor(em, et, mm.broadcast_to([128, 2, 512]),
                                        ALU.mult)
                et = em
            ets[j] = et

        def emit_pv(j):
            hp, ki = steps[j]
            pos = j % nk
            hA, hB = 2 * hp, 2 * hp + 1
            et = ets.pop(j)
            if pos == 0:
                otiles[hp] = (
                    o_psum.tile([65, 512], F32, tag="oa", name=f"oa{hp}"),
                    o_psum.tile([65, 512], F32, tag="ob", name=f"ob{hp}"),
                )
            oA, oB = otiles[hp]
            va = v_all[:, ki, hA * 65:hA * 65 + 65]
            vb = v_all[:, ki, hB * 65:hB * 65 + 65]
            nc.tensor.matmul(oA, va, et[:, 0, :],
                             start=(pos == 0), stop=(pos == nk - 1),
                             skip_group_check=True)
            nc.tensor.matmul(oB, vb, et[:, 1, :],
                             start=(pos == 0), stop=(pos == nk - 1),
                             skip_group_check=True)
            if pos == nk - 1:
                # fast-evict the accumulators to SBUF (frees the PSUM bank
                # for the next hp), then normalize off the critical path
                oS = dn_pool.tile([65, 2, 512], F32, tag="oS")
                nc.vector.tensor_copy(oS[:, 0, :], oA)
                nc.vector.tensor_copy(oS[:, 1, :], oB)
                dn = dn_pool.tile([1, 2, 512], F32, tag="dn")
                nc.vector.tensor_copy(dn, oS[64:65, :, :])
                rv = dn_pool.tile([1, 2, 512], F32, tag="rv")
                nc.vector.reciprocal_approx_fast(out=rv, in_=dn)
                rb = rb_pool.tile([128, 2, 512], F32, tag="rb")
                nc.gpsimd.partition_broadcast(rb, rv)
                nc.vector.tensor_tensor(oT_all[0:64, hp, q0:q0 + 512],
                                        oS[0:64, 0, :], rb[0:64, 0, :],
                                        ALU.mult)
                nc.vector.tensor_tensor(oT_all[64:128, hp, q0:q0 + 512],
                                        oS[0:64, 1, :], rb[0:64, 1, :],
                                        ALU.mult)

        # filler positions: spread evenly across the step list
        nf = len(fillers)
        fill_at = {}
        if nf:
            for fi in range(nf):
                pos = 1 + (fi * len(steps)) // nf
                fill_at.setdefault(pos, []).append(fillers[fi])

        for j in range(len(steps) + VPIPE):
            if j >= VPIPE:
                emit_pv(j - VPIPE)
            for f in fill_at.get(j, ()):
                f()
            if j < len(steps):
                emit_s(j)
            if j >= EPIPE and j - EPIPE < len(steps):
                emit_e(j - EPIPE)

    # ---- schedule ------------------------------------------------------
    # seg1: phase-1 tt0 in two c-passes (pass A runs while the c>=4 input
    # chunks are still in flight, keeping the PE fed from ~3us), then tt1
    for g in p1_groups(0, [nc.scalar], cs=(0, 4)):
        g()
    for g in p1_groups(0, [nc.vector], cs=(4, 8), accum=True):
        g()
    for g in p1_groups(1, [nc.vector, nc.scalar]):
        g()
    # seg2: qt0 attention, phase-1 tt2 as filler
    run_qt(0, p1_groups(2, [nc.vector]))
    # seg3: qt1 attention, phase-1 tt3 as filler
    run_qt(1, p1_groups(3, [nc.vector]))
    # seg4: qt2 attention, phase-3 for qt0/qt1 as filler
    run_qt(2, p3_groups(0) + p3_groups(1))
    # seg5: qt3 attention, phase-3 for qt2 as filler
    run_qt(3, p3_groups(2))
    # seg6: phase-3 for qt3
    for g in p3_groups(3):
        g()


def build_module():
    nc = bacc.Bacc("TRN2", target_bir_lowering=False, debug=False,
                   num_devices=NCORES)
    xT_d = nc.dram_tensor("xT", [D, T], BF16, kind="ExternalInput").ap()
    wT_d = nc.dram_tensor("wT", [D, 3 * JL], BF16, kind="ExternalInput").ap()
    woT_d = nc.dram_tensor("woT", [JL, D], BF16, kind="ExternalInput").ap()
    pfx_d = nc.dram_tensor("pfx", [1, 1], F32, kind="ExternalInput").ap()
    y_d = nc.dram_tensor("y", [T, D], F32, kind="ExternalOutput").ap()
    with tile.TileContext(nc) as tc:
        with ExitStack() as ctx:
            _emit(ctx, tc, y_d, xT_d, wT_d, woT_d, pfx_d)
    nc.compile()
    return nc


_NC = None


def _get_nc():
    global _NC
    if _NC is None:
        _NC = build_module()
    return _NC


def _bf16(a):
    import ml_dtypes
    return np.ascontiguousarray(a, dtype=np.float32).astype(ml_dtypes.bfloat16)


def shard_inputs(x, prefix_lengths, W_qkv, W_o):
    x = np.asarray(x, dtype=np.float32)
    W_qkv = np.asarray(W_qkv, dtype=np.float32)
    W_o = np.asarray(W_o, dtype=np.float32)
    pl = np.asarray(prefix_lengths)
    in_maps = []
    for c in range(NCORES):
        b, g = c // 2, c % 2
        W_loc = np.concatenate([
            W_qkv[JL * g:JL * (g + 1)],
            W_qkv[D + JL * g:D + JL * (g + 1)],
            W_qkv[2 * D + JL * g:2 * D + JL * (g + 1)],
        ], axis=0)
        p = float(min(max(int(pl[b]), 0), T))
        in_maps.append({
            "xT": _bf16(x[b].T),
            "wT": _bf16(W_loc.T),
            "woT": _bf16(W_o[:, JL * g:JL * (g + 1)].T),
            "pfx": np.array([[p]], dtype=np.float32),
        })
    return in_maps


def run(x, prefix_lengths, W_qkv, W_o, **kw):
    """Run the kernel; returns (y, BassKernelResults)."""
    nc = _get_nc()
    in_maps = shard_inputs(x, prefix_lengths, W_qkv, W_o)
    res = run_bass_kernel_spmd(nc, in_maps, core_ids=list(range(NCORES)), **kw)
    y = np.zeros((B, T, D), dtype=np.float32)
    for c in range(NCORES):
        y[c // 2] += res.results[c]["y"]
    return y, res


def kernel(x, prefix_lengths, W_qkv, W_o):
    y, _ = run(x, prefix_lengths, W_qkv, W_o)
    return y


# revision 22
# speedup vs baseline: 1.4403x; 1.0647x over previous
"""Trainium2 Bass kernel: prefix-LM causal self-attention (B=4, T=2048, D=1024, H=16).

Sharding: 8 cores = 4 batches x 2 head-groups (8 heads each).  Each core
computes QKV projection, masked attention and the output projection for its
(batch, head-group); the two partial output-projection results per batch are
summed on the host (tensor-parallel unshard).

v2 vs baseline:
  - bf16 activations/weights everywhere (rel err ~6e-3, tolerance 2e-2).
  - x and W fully resident in SBUF; DMA chunked+interleaved so the PE
    starts ~3us in and never starves (keeps the 2.4GHz p-state).
  - Diagonal-tile mask bias is pre-seeded into PSUM (gpsimd) before the
    S matmul accumulates on top, so the S->exp->PV chain never waits on
    a DVE mask add.
  - Phases are interleaved: attention for q-tile qt starts as soon as
    its K/V rows exist (after phase-1 tt<=1); phase-1 tt2/tt3 and the
    phase-3 output projection are spread between attention steps as PE
    filler so the exp (ACT) stream stays hidden under PE work.

Mask identity: allowed(q, k) <=> k <= max(q, p-1), i.e. blocked <=>
(k > q) AND (k >= p).  Scores are computed transposed S^T[k, q] so the
softmax denominator comes free from a ones-augmented V column in the P@V
matmul (no row-max subtraction needed; |scores| ~ N(0,1)).
"""

import sys

for _p in ("/opt/trn_rl_repo", "/opt/pypackages"):
    if _p not in sys.path:
        sys.path.append(_p)

from contextlib import ExitStack

import numpy as np

import concourse.bass as bass  # noqa: F401
import concourse.tile as tile
from concourse import bacc, mybir
from concourse.bass_utils import run_bass_kernel_spmd

F32 = mybir.dt.float32
BF16 = mybir.dt.bfloat16
EXP = mybir.ActivationFunctionType.Exp
ALU = mybir.AluOpType

B, T, D, H, DK = 4, 2048, 1024, 16, 64
HL = 8            # heads per core
JL = HL * DK      # 512 local attention dims
NCORES = 8
NEG = -1.0e30
SCALE = 0.125     # 1/sqrt(dk)
# Static per-q-tile key extent (prefix_lengths < 1024 by construction).
KMAX = [1024, 1024, 1536, 2048]
def _emit(ctx: ExitStack, tc, y_d, xT_d, wT_d, woT_d, pfx_d):
    nc = tc.nc

    # ---- constants ----------------------------------------------------
    const = ctx.enter_context(tc.tile_pool(name="const", bufs=1))
    ones_t = const.tile([128, 512], F32)
    nc.vector.memset(ones_t, 1.0)
    # tri_all[:, i, :][r, j] = 1.0 where (128*i + r) > j else 0  (k > q)
    tri_all = const.tile([128, 4, 512], F32)
    for i in range(4):
        nc.gpsimd.affine_select(
            tri_all[:, i, :], ones_t, pattern=[[-1, 512]], base=128 * i,
            channel_multiplier=1, compare_op=ALU.is_gt, fill=0.0,
        )
    kio = const.tile([128, 1], F32)
    nc.gpsimd.iota(kio, pattern=[[1, 1]], base=0, channel_multiplier=1,
                   allow_small_or_imprecise_dtypes=True)
    p_one = const.tile([1, 1], F32)
    nc.sync.dma_start(p_one, pfx_d)
    p_bcast = const.tile([128, 1], F32)
    nc.gpsimd.partition_broadcast(p_bcast, p_one)

    # ---- resident tensors ---------------------------------------------
    big = ctx.enter_context(tc.tile_pool(name="big", bufs=1))
    xt_all = big.tile([128, 8, T], BF16)          # x^T, chunked by c
    w_all = big.tile([128, 8, 3 * JL], BF16)      # W_qkv^T, chunked by c
    qk_all = big.tile([128, 8, T], BF16)          # rows: q(0-3) then k(4-7)
    v_all = big.tile([128, 16, HL * 65], BF16)    # per head 64 v-dims + ones col
    wo_all = big.tile([128, 4, D], BF16)          # W_o^T, chunked by jc
    oT_all = big.tile([128, 4, T], BF16)          # attention out^T per hp

    # ones columns for the softmax denominator
    ones_cols = v_all.rearrange("p t (h e) -> p t h e", e=65)[:, :, :, 64:65]
    nc.vector.memset(ones_cols, 1.0)

    # ---- input DMAs (ordered: w/x interleaved per c, then wo) ---------
    for c in range(8):
        nc.sync.dma_start(w_all[:, c, :], wT_d[c * 128:(c + 1) * 128, :])
        nc.sync.dma_start(xt_all[:, c, :], xT_d[c * 128:(c + 1) * 128, :])
    for jc in range(4):
        nc.sync.dma_start(wo_all[:, jc, :], woT_d[jc * 128:(jc + 1) * 128, :])

    # ---- shared pools --------------------------------------------------
    psf = ctx.enter_context(tc.tile_pool(name="psf", bufs=2, space="PSUM"))
    s_psum = ctx.enter_context(tc.tile_pool(name="ps2", bufs=2, space="PSUM"))
    o_psum = ctx.enter_context(tc.tile_pool(name="po", bufs=1, space="PSUM"))
    evict_pool = ctx.enter_context(tc.tile_pool(name="ev", bufs=4))
    bias_pool = ctx.enter_context(tc.tile_pool(name="bias", bufs=2))
    pv_pool = ctx.enter_context(tc.tile_pool(name="pv", bufs=12))
    e_pool = ctx.enter_context(tc.tile_pool(name="e", bufs=4))
    dn_pool = ctx.enter_context(tc.tile_pool(name="dn", bufs=2))
    rb_pool = ctx.enter_context(tc.tile_pool(name="rb", bufs=2))
    y_pool = ctx.enter_context(tc.tile_pool(name="ysb", bufs=4))

    # ---- phase-1 group emitters ---------------------------------------
    def _evict(eng, dst, src, accum=False):
        if accum:
            nc.vector.tensor_tensor(dst, src, dst, ALU.add)
        elif eng is nc.scalar:
            eng.copy(dst, src)
        else:
            eng.tensor_copy(dst, src)

    def p1_qk_group(tt, ot, eng, cs=(0, 8), accum=False):
        t0 = tt * 512
        c0, c1 = cs
        ps = psf.tile([128, 512], F32, tag="psf")
        for c in range(c0, c1):
            nc.tensor.matmul(ps, w_all[:, c, ot * 128:(ot + 1) * 128],
                             xt_all[:, c, t0:t0 + 512],
                             start=(c == c0), stop=(c == c1 - 1))
        _evict(eng, qk_all[:, ot, t0:t0 + 512], ps, accum)

    def p1_v_group(tt, st, eng, cs=(0, 8), accum=False):
        kt = tt * 4 + st
        t0 = tt * 512 + st * 128
        c0, c1 = cs
        ps = psf.tile([128, 512], F32, tag="psf")
        for c in range(c0, c1):
            nc.tensor.matmul(ps, xt_all[:, c, t0:t0 + 128],
                             w_all[:, c, 2 * JL:3 * JL],
                             start=(c == c0), stop=(c == c1 - 1))
        dst = v_all[:, kt, :].rearrange("p (h e) -> p h e", h=HL)[:, :, 0:64]
        _evict(eng, dst, ps.rearrange("p (h e) -> p h e", h=HL), accum)

    def p1_groups(tt, evict_engines, cs=(0, 8), accum=False):
        out = []
        for ot in range(8):
            eng = evict_engines[ot % len(evict_engines)]
            out.append(lambda ot=ot, eng=eng: p1_qk_group(tt, ot, eng, cs, accum))
        for st in range(4):
            eng = evict_engines[st % len(evict_engines)]
            out.append(lambda st=st, eng=eng: p1_v_group(tt, st, eng, cs, accum))
        return out

    # ---- phase-3 group emitters ---------------------------------------
    def p3_group(ttt, ob):
        t0 = ttt * 128
        ps = psf.tile([128, 512], F32, tag="psf")
        for jc in range(4):
            nc.tensor.matmul(ps, oT_all[:, jc, t0:t0 + 128],
                             wo_all[:, jc, ob * 512:(ob + 1) * 512],
                             start=(jc == 0), stop=(jc == 3))
        ysb = y_pool.tile([128, 512], F32, tag="ysb")
        nc.vector.tensor_copy(ysb, ps)
        nc.sync.dma_start(y_d[t0:t0 + 128, ob * 512:(ob + 1) * 512], ysb)

    def p3_groups(qt):
        return [lambda ttt=qt * 4 + i, ob=ob: p3_group(ttt, ob)
                for i in range(4) for ob in range(2)]

    # ---- attention: one q-tile, with PE filler interleaved ------------
    def run_qt(qt, fillers, evict_eng=None, last_fast=False):
        q0 = qt * 512
        nk = KMAX[qt] // 128
        # per-(k-row-tile) prefix bias vectors: (row_k >= p) * NEG
        k0s = [q0 + 128 * i for i in range(4)]
        if qt == 0:
            k0s += [512, 640, 768, 896]
        pb = {}
        pks = {}
        for k0 in k0s:
            pk = pv_pool.tile([128, 1], F32, tag="pk")
            nc.vector.tensor_scalar_add(pk, p_bcast, float(-k0))  # p - k0
            pv = pv_pool.tile([128, 1], F32, tag="pv")
            nc.vector.tensor_scalar(pv, kio, pk, NEG, ALU.is_ge, ALU.mult)
            pb[k0] = pv
            pks[k0] = pk
        # diagonal 0/1 keep-masks: 1 - tri(k>q)*(k>=p)
        mask_all = bias_pool.tile([128, 4, 512], BF16, tag="mask")
        for i in range(4):
            pm = pv_pool.tile([128, 1], F32, tag="pm")
            nc.vector.tensor_scalar(pm, kio, pks[q0 + 128 * i], -1.0,
                                    ALU.is_ge, ALU.mult)
            nc.vector.tensor_scalar(mask_all[:, i, :], tri_all[:, i, :], pm,
                                    1.0, ALU.mult, ALU.add)

        # spread diagonal (masked) k-tiles out in the step order
        def kt_order(nk):
            diag = [k for k in range(nk) if 0 <= k * 128 - q0 <= 384]
            rest = [k for k in range(nk) if k not in diag]
            out, di, ri = [], 0, 0
            for j in range(nk):
                if j % 4 == 0 and di < len(diag):
                    out.append(diag[di]); di += 1
                elif ri < len(rest):
                    out.append(rest[ri]); ri += 1
                else:
                    out.append(diag[di]); di += 1
            return out

        korder = kt_order(nk)
        steps = [(hp, ki) for hp in range(4) for ki in korder]
        EPIPE = 1   # exp stage lags S by this many steps
        VPIPE = 3   # PV stage lags S
        live = {}
        ets = {}
        otiles = {}

        def emit_s(j):
            hp, ki = steps[j]
            k0 = ki * 128
            sps = s_psum.tile([128, 2, 512], F32, tag="s")
            qa = qk_all[0:64, hp, q0:q0 + 512]
            qb = qk_all[64:128, hp, q0:q0 + 512]
            ka = qk_all[0:64, 4 + hp, k0:k0 + 128]
            kb = qk_all[64:128, 4 + hp, k0:k0 + 128]
            nc.tensor.matmul(sps[:, 0, :], ka, qa, start=True, stop=True)
            nc.tensor.matmul(sps[:, 1, :], kb, qb, start=True, stop=True)
            live[j] = sps

        def emit_e(j):
            hp, ki = steps[j]
            k0 = ki * 128
            rel = k0 - q0
            sps = live.pop(j)
            et = e_pool.tile([128, 2, 512], BF16, tag="e")
            if rel > 384:        # fully above diagonal: prefix-only rows
                nc.scalar.activation(et, sps, EXP, bias=pb[k0], scale=SCALE)
            else:
                nc.scalar.activation(et, sps, EXP, scale=SCALE)
            if 0 <= rel <= 384:  # diagonal: zero the blocked region post-exp
                mm = mask_all[:, rel // 128, :].unsqueeze(1)
                em = e_pool.tile([128, 2, 512], BF16, tag="em")
                nc.vector.tensor_tensor(em, et, mm.broadcast_to([128, 2, 512]),
                                        ALU.mult)
                et = em
            ets[j] = et

        def emit_pv(j):
            hp, ki = steps[j]
            pos = j % nk
            hA, hB = 2 * hp, 2 * hp + 1
            et = ets.pop(j)
            if pos == 0:
                otiles[hp] = (
                    o_psum.tile([65, 512], F32, tag="oa", name=f"oa{hp}"),
                    o_psum.tile([65, 512], F32, tag="ob", name=f"ob{hp}"),
                )
            oA, oB = otiles[hp]
            va = v_all[:, ki, hA * 65:hA * 65 + 65]
            vb = v_all[:, ki, hB * 65:hB * 65 + 65]
            nc.tensor.matmul(oA, va, et[:, 0, :],
                             start=(pos == 0), stop=(pos == nk - 1),
                             skip_group_check=True)
            nc.tensor.matmul(oB, vb, et[:, 1, :],
                             start=(pos == 0), stop=(pos == nk - 1),
                             skip_group_check=True)
            if pos == nk - 1:
                # fast-evict the accumulators to SBUF (frees the PSUM bank
                # for the next hp), then normalize off the critical path
                oS = dn_pool.tile([65, 2, 512], F32, tag="oS")
                _evict(evict_eng or nc.vector, oS[:, 0, :], oA)
                _evict(evict_eng or nc.vector, oS[:, 1, :], oB)
                dn = dn_pool.tile([1, 2, 512], F32, tag="dn")
                nc.vector.tensor_copy(dn, oS[64:65, :, :])
                rv = dn_pool.tile([1, 2, 512], F32, tag="rv")
                nc.vector.reciprocal_approx_fast(out=rv, in_=dn)
                if last_fast and hp == 3:
                    # latency-critical final head-pair: broadcast 1/denom
                    # across partitions on the PE (ones-stationary matmul)
                    # instead of the slow gpsimd partition_broadcast
                    rp = s_psum.tile([128, 2, 512], F32, tag="s")
                    nc.tensor.matmul(rp[:, 0, :], ones_t[0:1, 0:128],
                                     rv[:, 0, :])
                    nc.tensor.matmul(rp[:, 1, :], ones_t[0:1, 0:128],
                                     rv[:, 1, :])
                    rba, rbb = rp[0:64, 0, :], rp[0:64, 1, :]
                else:
                    rb = rb_pool.tile([128, 2, 512], F32, tag="rb")
                    nc.gpsimd.partition_broadcast(rb, rv)
                    rba, rbb = rb[0:64, 0, :], rb[0:64, 1, :]
                nc.vector.tensor_tensor(oT_all[0:64, hp, q0:q0 + 512],
                                        oS[0:64, 0, :], rba, ALU.mult)
                nc.vector.tensor_tensor(oT_all[64:128, hp, q0:q0 + 512],
                                        oS[0:64, 1, :], rbb, ALU.mult)

        # filler positions: spread evenly across the step list
        nf = len(fillers)
        fill_at = {}
        if nf:
            for fi in range(nf):
                pos = 1 + (fi * len(steps)) // nf
                fill_at.setdefault(pos, []).append(fillers[fi])

        for j in range(len(steps) + VPIPE):
            if j >= VPIPE:
                emit_pv(j - VPIPE)
            for f in fill_at.get(j, ()):
                f()
            if j < len(steps):
                emit_s(j)
            if j >= EPIPE and j - EPIPE < len(steps):
                emit_e(j - EPIPE)

    # ---- schedule ------------------------------------------------------
    # seg1: phase-1 tt0 in two c-passes (pass A runs while the c>=4 input
    # chunks are still in flight, keeping the PE fed from ~3us), then tt1
    for g in p1_groups(0, [nc.scalar], cs=(0, 4)):
        g()
    for g in p1_groups(0, [nc.vector], cs=(4, 8), accum=True):
        g()
    for g in p1_groups(1, [nc.vector, nc.scalar]):
        g()
    # seg2: qt0 attention, phase-1 tt2 as filler (ACT has slack here, so
    # it takes the o-accumulator evictions)
    run_qt(0, p1_groups(2, [nc.vector]), evict_eng=nc.scalar)
    # seg3: qt1 attention, phase-1 tt3 as filler
    run_qt(1, p1_groups(3, [nc.vector]), evict_eng=nc.scalar)
    # seg4: qt2 attention (ACT-heavy), phase-3 for qt0 as filler
    run_qt(2, p3_groups(0))
    # seg5: qt3 attention, phase-3 for qt1/qt2 as filler
    run_qt(3, p3_groups(1) + p3_groups(2), last_fast=True)
    # seg6: phase-3 for qt3
    for g in p3_groups(3):
        g()


def build_module():
    nc = bacc.Bacc("TRN2", target_bir_lowering=False, debug=False,
                   num_devices=NCORES)
    xT_d = nc.dram_tensor("xT", [D, T], BF16, kind="ExternalInput").ap()
    wT_d = nc.dram_tensor("wT", [D, 3 * JL], BF16, kind="ExternalInput").ap()
    woT_d = nc.dram_tensor("woT", [JL, D], BF16, kind="ExternalInput").ap()
    pfx_d = nc.dram_tensor("pfx", [1, 1], F32, kind="ExternalInput").ap()
    y_d = nc.dram_tensor("y", [T, D], F32, kind="ExternalOutput").ap()
    with tile.TileContext(nc) as tc:
        with ExitStack() as ctx:
            _emit(ctx, tc, y_d, xT_d, wT_d, woT_d, pfx_d)
    nc.compile()
    return nc


_NC = None


def _get_nc():
    global _NC
    if _NC is None:
        _NC = build_module()
    return _NC


def _bf16(a):
    import ml_dtypes
    return np.ascontiguousarray(a, dtype=np.float32).astype(ml_dtypes.bfloat16)


def shard_inputs(x, prefix_lengths, W_qkv, W_o):
    x = np.asarray(x, dtype=np.float32)
    W_qkv = np.asarray(W_qkv, dtype=np.float32)
    W_o = np.asarray(W_o, dtype=np.float32)
    pl = np.asarray(prefix_lengths)
    in_maps = []
    for c in range(NCORES):
        b, g = c // 2, c % 2
        W_loc = np.concatenate([
            W_qkv[JL * g:JL * (g + 1)],
            W_qkv[D + JL * g:D + JL * (g + 1)],
            W_qkv[2 * D + JL * g:2 * D + JL * (g + 1)],
        ], axis=0)
        p = float(min(max(int(pl[b]), 0), T))
        in_maps.append({
            "xT": _bf16(x[b].T),
            "wT": _bf16(W_loc.T),
            "woT": _bf16(W_o[:, JL * g:JL * (g + 1)].T),
            "pfx": np.array([[p]], dtype=np.float32),
        })
    return in_maps


def run(x, prefix_lengths, W_qkv, W_o, **kw):
    """Run the kernel; returns (y, BassKernelResults)."""
    nc = _get_nc()
    in_maps = shard_inputs(x, prefix_lengths, W_qkv, W_o)
    res = run_bass_kernel_spmd(nc, in_maps, core_ids=list(range(NCORES)), **kw)
    y = np.zeros((B, T, D), dtype=np.float32)
    for c in range(NCORES):
        y[c // 2] += res.results[c]["y"]
    return y, res


def kernel(x, prefix_lengths, W_qkv, W_o):
    y, _ = run(x, prefix_lengths, W_qkv, W_o)
    return y
